# revision 9
# baseline (speedup 1.0000x reference)
"""TRN2 Bass/Tile kernel: BatchNorm1d + 4-head self-attention + out-projection.

Reference computation (b=4, c=256, n=4096, heads=4, d=64):
    xn   = BN(x)  (training-mode stats over batch+length)
    qkv  = w_qkv @ xn ;  q,k,v  (q scaled by d^-0.5)
    out  = softmax(q^T k) @ v^T  per (batch, head)
    y    = w_out @ out + b_out

Sharding over 8 NeuronCores: core i handles (batch i//2, query-half i%2).
Keys/values are processed in the core-local order [mine, other] (softmax and
attention are invariant to key permutation).

Design notes (v2 rewrite over the AllReduce-free baseline):
  - NO cross-core collective: every core receives the other 3 batches
    (fp8 for stats only) and computes the EXACT global BN statistics
    locally (own batch via DVE bn_stats, part of the rest via ACT
    Copy/Square accum_out sums).
  - BN scale folded into the QKV weights; shift becomes per-channel
    biases.  k-projection bias dropped (per-query score shift, softmax
    cancels it).  v-projection bias folded into the OUTPUT bias:
    W_out @ (W_v @ shift) is a per-output-channel constant, computed
    with tiny PE matmuls at startup -> zero steady-state cost.
  - HEAD-PAIR layout: q/k stored [128, hp, n] with head 2hp on
    partitions 0:64 and head 2hp+1 on 64:128 (no zero padding).  The
    scores for both heads of a pair run as TWO CONCURRENT K=64
    row-tiled matmuls (tile_position (0,0)/(64,0)) - measured 1.88x
    the serial K=128 rate on HW.
  - exp split 50/50: even key-chunks on ACT (table exp), odd chunks on
    the DVE as a Schraudolph bf16 bit-trick (fused mult+add to int16).
  - AV: lhsT = vT-block [128key, 65] bf16 (64 v channels + ones column
    -> softmax denominator for free); attn kept per-head at partitions
    0:64 (attn2 [64, h, n]); out-projection runs as 4 accumulating
    K=64 matmuls per 128-channel block (streaming time only depends on
    the moving size, so this costs ~nothing vs K=128).
  - Normalization (deferred into the next pair): ACT/DVE copy the AV
    PSUM to SBUF, DVE reciprocal_approx_fast in place, gpsimd
    partition_broadcast, one DVE multiply for both heads.
  - PSUM: spool 2x[128,2,512] (scores pairs, outproj, startup QKV) +
    avpool 4x[128,512] (AV accumulators + fused-phase k/v psum) = 8
    banks exactly.
  - k/v projections are fused INTO the first attention pair (produced
    just-in-time, one key-chunk ahead), so ACT/DVE exp work starts
    ~10us earlier; a short burst of keep-warm PE matmuls sequenced
    right before the stats-combine keeps the HAM clock at 8/8 when the
    real matmuls arrive.
"""

import numpy as np

import concourse.bacc as bacc
import concourse.tile as tile
from concourse import mybir
from concourse.bass_utils import run_bass_kernel_spmd

B, C, N = 4, 256, 4096
H, D = 4, 64
P = 128
CT = C // P            # 2 channel tiles of 128
RB = 2                 # row blocks for q/k rows (256 = 2*128)
HP = 2                 # head pairs
NH = N // 2            # 2048 queries per core
QS = 512               # query subtile (1 PSUM bank of fp32)
NQS = NH // QS         # 4
KC = 128               # key chunk (matmul stationary width)
NKC = N // KC          # 32
EPS = 1e-5
SCALE = D ** -0.5
F32 = mybir.dt.float32
BF16 = mybir.dt.bfloat16
XDT = BF16
F8 = mybir.dt.float8e4
NCORES = 8
WARM = 22          # keep-warm PE matmuls issued right before stats-combine
SCH_A = 184.6650244    # 2^7 / ln 2
SCH_B = 16250.65       # 127*128 - c_opt (half-way rounding compensation)


def _body(tc, x_mine, x_other, x_rest, w_qkvT, w_oT2, bn_w, bn_b, b_out, out):
    from contextlib import ExitStack

    nc = tc.nc
    AF = mybir.ActivationFunctionType
    OP = mybir.AluOpType

    with ExitStack() as ctx:
        big = ctx.enter_context(tc.tile_pool(name="big", bufs=1))
        small = ctx.enter_context(tc.tile_pool(name="small", bufs=1))
        epool = ctx.enter_context(tc.tile_pool(name="epool", bufs=4))
        oupool = ctx.enter_context(tc.tile_pool(name="oupool", bufs=2))
        rpool = ctx.enter_context(tc.tile_pool(name="rpool", bufs=2))
        opool = ctx.enter_context(tc.tile_pool(name="opool", bufs=2))
        spool = ctx.enter_context(tc.tile_pool(name="spool", bufs=2, space="PSUM"))
        avpool = ctx.enter_context(tc.tile_pool(name="avpool", bufs=2, space="PSUM"))
        kvpool = ctx.enter_context(tc.tile_pool(name="kvpool", bufs=2, space="PSUM"))

        # ---- loads: x_mine first (BN stats critical path) ---------------
        xn_sb = big.tile([P, CT, N], XDT, tag="xn")  # RAW x, key order [mine|other]
        xm_r = x_mine.rearrange("(ct p) n -> p ct n", p=P)
        for ct in range(CT):
            for half in range(2):
                nc.sync.dma_start(
                    out=xn_sb[:, ct, half * (NH // 2) : (half + 1) * (NH // 2)],
                    in_=xm_r[:, ct, half * (NH // 2) : (half + 1) * (NH // 2)],
                )
        nc.sync.dma_start(
            out=xn_sb[:, :, NH:N], in_=x_other.rearrange("(ct p) n -> p ct n", p=P)
        )
        wq_sb = big.tile([P, CT, 3 * C], XDT)
        nc.sync.dma_start(
            out=wq_sb, in_=w_qkvT.rearrange("(ct p) o -> p ct o", p=P)
        )
        wo2_sb = big.tile([D, H, C], XDT)   # w_out^T as [d, h, o]
        nc.sync.dma_start(out=wo2_sb, in_=w_oT2)
        bnw_sb = small.tile([P, CT, 1], F32)
        nc.sync.dma_start(out=bnw_sb, in_=bn_w)
        bnb_sb = small.tile([P, CT, 1], F32)
        nc.sync.dma_start(out=bnb_sb, in_=bn_b)
        bo_sb = small.tile([P, RB, 1], F32)
        nc.sync.dma_start(out=bo_sb, in_=b_out)

        # ---- BN stats: EXACT global stats computed locally --------------
        NRC = 4            # x_rest DMA chunks per batch (1024 fp8 cols)
        RCW = N // NRC
        SG = N // 512      # own-batch 512-col stat groups per ct
        NCH = 3 * NRC
        ACT_CH = (1, 3, 5, 7, 9, 11)   # chunks reduced on ACT via accum sums
        NACT = len(ACT_CH)
        RG = RCW // 512
        NDVE_R = (NCH - NACT) * RG
        NREC = SG + NDVE_R         # bn_stats records per ct
        NS = N + NDVE_R * 512      # samples covered by bn_stats records
        NT = B * N                 # total samples per channel
        stat6 = small.tile([P, CT, NREC, 6], F32)
        for ct in range(CT):
            xm = xn_sb[:, ct, :].rearrange("p (s f) -> p s f", f=512)
            for s in range(SG):
                nc.vector.bn_stats(out=stat6[:, ct, s, :], in_=xm[:, s, :])
        stg = ctx.enter_context(tc.tile_pool(name="stg", bufs=4))
        trash = small.tile([P, RCW], BF16)
        acc_x = small.tile([P, CT, NACT], F32)
        acc_x2 = small.tile([P, CT, NACT], F32)
        ci_dve = 0
        ci_act = 0
        for rb_ in range(3):
            for chunk in range(NRC):
                st = stg.tile([P, CT, RCW], F8, tag="stg")
                nc.sync.dma_start(out=st, in_=x_rest[rb_, chunk])
                if rb_ * NRC + chunk in ACT_CH:
                    # ACT computes plain sums: Sum(x) via Copy-accumulate,
                    # Sum(x^2) via Square-accumulate
                    for ct in range(CT):
                        nc.scalar.activation(
                            out=trash, in_=st[:, ct, :], func=AF.Copy,
                            accum_out=acc_x[:, ct, ci_act : ci_act + 1],
                        )
                        nc.scalar.activation(
                            out=trash, in_=st[:, ct, :], func=AF.Square,
                            accum_out=acc_x2[:, ct, ci_act : ci_act + 1],
                        )
                    ci_act += 1
                else:
                    for ct in range(CT):
                        xr = st[:, ct, :].rearrange("p (s f) -> p s f", f=512)
                        for s in range(RG):
                            nc.vector.bn_stats(
                                out=stat6[:, ct, SG + ci_dve * RG + s, :],
                                in_=xr[:, s, :],
                            )
                    ci_dve += 1

        # ---- PE keep-warm ------------------------------------------------
        # The HAM activity monitor runs the PE at half clock until it has
        # seen a ~3.4us busy window.  A dense burst of N=512 matmuls gated
        # (via dum2, written by the DVE just before the combine chain) to
        # run during the stats-combine warms the clock exactly in time for
        # the QKV projections.
        dum2 = small.tile([1, QS], BF16)
        nc.vector.memset(dum2, 1.0)
        for i in range(WARM):
            scrap = kvpool.tile([P, QS], F32, tag="kv", name="scrap")
            nc.tensor.matmul(
                out=scrap[0:1, :], lhsT=dum2[0:1, 0:1], rhs=dum2,
                start=True, stop=True,
            )

        mv = small.tile([P, CT, 2], F32)
        for ct in range(CT):
            nc.vector.bn_aggr(out=mv[:, ct, :], in_=stat6[:, ct])
        # combine: totals = bn_aggr subset (NS samples) + ACT sums
        sum_t = small.tile([P, CT, 1], F32)
        nc.vector.tensor_reduce(
            out=sum_t, in_=acc_x, axis=mybir.AxisListType.X,
            op=mybir.AluOpType.add,
        )
        sq_t = small.tile([P, CT, 1], F32)
        nc.vector.tensor_reduce(
            out=sq_t, in_=acc_x2, axis=mybir.AxisListType.X,
            op=mybir.AluOpType.add,
        )
        msq_s = small.tile([P, CT, 1], F32)
        nc.vector.tensor_mul(out=msq_s, in0=mv[:, :, 0:1], in1=mv[:, :, 0:1])
        e2_s = small.tile([P, CT, 1], F32)
        nc.vector.tensor_add(out=e2_s, in0=mv[:, :, 1:2], in1=msq_s)
        # sum_t += mean_s * NS ; sq_t += e2_s * NS
        tmp_s = small.tile([P, CT, 1], F32)
        nc.vector.tensor_scalar_mul(out=tmp_s, in0=mv[:, :, 0:1], scalar1=float(NS))
        nc.vector.tensor_add(out=sum_t, in0=sum_t, in1=tmp_s)
        nc.vector.tensor_scalar_mul(out=tmp_s, in0=e2_s, scalar1=float(NS))
        nc.vector.tensor_add(out=sq_t, in0=sq_t, in1=tmp_s)
        mvg = small.tile([P, CT, 2], F32)
        nc.vector.tensor_scalar_mul(
            out=mvg[:, :, 0:1], in0=sum_t, scalar1=1.0 / NT
        )
        nc.vector.tensor_scalar_mul(out=tmp_s, in0=sq_t, scalar1=1.0 / NT)
        nc.vector.tensor_mul(
            out=mvg[:, :, 1:2], in0=mvg[:, :, 0:1], in1=mvg[:, :, 0:1]
        )
        nc.vector.tensor_sub(out=mvg[:, :, 1:2], in0=tmp_s, in1=mvg[:, :, 1:2])
        mv = mvg

        eps_sb = small.tile([P, 1], F32)
        nc.vector.memset(eps_sb, EPS)

        # ---- global mean/var -> s = bn_w * rstd, shift = bn_b - mean*s --
        mean_g = mv[:, :, 0:1]
        var_g = mv[:, :, 1:2]
        sd = small.tile([P, CT, 1], F32)
        nc.scalar.activation(out=sd, in_=var_g, func=AF.Sqrt, bias=eps_sb)
        rstd = small.tile([P, CT, 1], F32)
        nc.vector.reciprocal(out=rstd, in_=sd)
        s_sb = small.tile([P, CT, 1], F32)
        nc.vector.tensor_mul(out=s_sb, in0=bnw_sb, in1=rstd)
        shift_sb = small.tile([P, CT, 1], F32)
        nc.vector.tensor_mul(out=shift_sb, in0=mean_g, in1=s_sb)
        nc.vector.tensor_sub(out=shift_sb, in0=bnb_sb, in1=shift_sb)
        shift_bf = small.tile([P, CT, 1], BF16)
        nc.vector.tensor_copy(out=shift_bf, in_=shift_sb)

        # ---- biases from the ORIGINAL weights ---------------------------
        # q bias qkb[:, rb] = W_q[rb] @ shift (k bias dropped: softmax
        # cancels a per-query score shift).
        # v bias per head as a [64,1] column: vb4[:, h] = W_v,h @ shift.
        # Output-bias correction: bo_eff = b_out + W_out @ vb  (the v bias
        # contributes attn-weight-sum * vb = vb after normalization).
        bias_ps = spool.tile([P, 2, QS], F32, tag="sp", name="bias")
        for rb in range(RB):
            for ct in range(CT):
                nc.tensor.matmul(
                    out=bias_ps[:, 0, rb : rb + 1],
                    lhsT=wq_sb[:, ct, rb * P : (rb + 1) * P],
                    rhs=shift_bf[:, ct],
                    start=(ct == 0),
                    stop=(ct == CT - 1),
                )
        for h in range(H):
            for ct in range(CT):
                nc.tensor.matmul(
                    out=bias_ps[0:D, 0, RB + h : RB + h + 1],
                    lhsT=wq_sb[:, ct, 2 * C + h * D : 2 * C + (h + 1) * D],
                    rhs=shift_bf[:, ct],
                    start=(ct == 0),
                    stop=(ct == CT - 1),
                )
        qkb_sb = small.tile([P, RB], F32)
        nc.vector.tensor_copy(out=qkb_sb, in_=bias_ps[:, 0, 0:RB])
        vb4 = small.tile([D, H], BF16)
        nc.vector.tensor_copy(out=vb4, in_=bias_ps[0:D, 0, RB : RB + H])
        # bo_eff = b_out + sum_h W_out[:, (h,:)] @ vb4[:, h]
        for rb in range(RB):
            for h in range(H):
                nc.tensor.matmul(
                    out=bias_ps[:, 1, rb : rb + 1],
                    lhsT=wo2_sb[:, h, rb * P : (rb + 1) * P],
                    rhs=vb4[:, h : h + 1],
                    start=(h == 0),
                    stop=(h == H - 1),
                )
        bo_eff = small.tile([P, RB], F32)
        nc.vector.tensor_add(
            out=bo_eff, in0=bias_ps[:, 1, 0:RB],
            in1=bo_sb.rearrange("p rb one -> p (rb one)"),
        )

        # fold diag(s) into the weights, q columns first so q matmuls can
        # start while k/v columns are still being scaled (d^-0.5 is folded
        # into the q columns host-side)
        for sec in range(3):
            for ct in range(CT):
                nc.vector.tensor_scalar_mul(
                    out=wq_sb[:, ct, sec * C : (sec + 1) * C],
                    in0=wq_sb[:, ct, sec * C : (sec + 1) * C],
                    scalar1=s_sb[:, ct],
                )

        # ---- attention state --------------------------------------------
        q2 = big.tile([P, HP, NH], BF16)
        k2 = big.tile([P, HP, N], BF16)
        vT = big.tile([P, NKC, H, D + 1], BF16)
        nc.gpsimd.memset(vT[:, :, :, D : D + 1], 1.0)
        attn2 = big.tile([D, H, NH], BF16)
        out_r = out.rearrange("(rb p) n -> p rb n", p=P)

        # ---- q projection (consumes RAW x; bias fused into copies) ------
        for j in range(NQS):
            ps = spool.tile([P, 2, QS], F32, tag="sp", name="qproj")
            for hp in range(HP):
                for ct in range(CT):
                    nc.tensor.matmul(
                        out=ps[:, hp, :],
                        lhsT=wq_sb[:, ct, hp * P : (hp + 1) * P],
                        rhs=xn_sb[:, ct, j * QS : (j + 1) * QS],
                        start=(ct == 0),
                        stop=(ct == CT - 1),
                    )
            nc.vector.tensor_scalar_add(
                out=q2[:, 0, j * QS : (j + 1) * QS], in0=ps[:, 0, :],
                scalar1=qkb_sb[:, 0:1],
            )
            nc.scalar.activation(
                out=q2[:, 1, j * QS : (j + 1) * QS], in_=ps[:, 1, :],
                func=AF.Identity, bias=qkb_sb[:, 1:2],
            )

        # ---- k/v production (fused into the first pair's chunk loop) ---
        def emit_kproj(hp, jk, eng):
            ps = kvpool.tile([P, QS], F32, tag="kv", name="kproj")
            for ct in range(CT):
                nc.tensor.matmul(
                    out=ps,
                    lhsT=wq_sb[:, ct, C + hp * P : C + (hp + 1) * P],
                    rhs=xn_sb[:, ct, jk * QS : (jk + 1) * QS],
                    start=(ct == 0),
                    stop=(ct == CT - 1),
                )
            dst = k2[:, hp, jk * QS : (jk + 1) * QS]
            if eng == "act":
                nc.scalar.activation(out=dst, in_=ps, func=AF.Identity)
            else:
                nc.vector.tensor_copy(out=dst, in_=ps)

        def emit_vproj(nb):
            ps = kvpool.tile([P, QS], F32, tag="kv", name="vproj")
            for ct in range(CT):
                nc.tensor.matmul(
                    out=ps[:, 0:C],
                    lhsT=xn_sb[:, ct, nb * KC : (nb + 1) * KC],
                    rhs=wq_sb[:, ct, 2 * C : 3 * C],
                    start=(ct == 0),
                    stop=(ct == CT - 1),
                )
            nc.vector.tensor_copy(
                out=vT[:, nb, :, 0:D],
                in_=ps[:, 0:C].rearrange("p (h d) -> p h d", d=D),
            )

        # ---- attention ---------------------------------------------------
        # Normalize/outproj of pair X are deferred into pair X+1 (hooks at
        # fixed chunk indices) so they never stall the exp engines.  The AV
        # matmuls run one chunk behind the scores (pend_av), carried across
        # pair boundaries.
        pend_av = [None]
        pend_norm = [None]

        def emit_av(p):
            e_, c_, avp_, hp_ = p
            for u in range(2):
                nc.tensor.matmul(
                    out=avp_[u][0 : D + 1, :],
                    lhsT=vT[:, c_, 2 * hp_ + u, :],
                    rhs=e_[:, u, :],
                    start=(c_ == 0),
                    stop=(c_ == NKC - 1),
                )

        def outproj(j_):
            ps = spool.tile([P, 2, QS], F32, tag="sp", name="oproj")
            for rb in range(RB):
                for h in range(H):
                    nc.tensor.matmul(
                        out=ps[:, rb, :],
                        lhsT=wo2_sb[:, h, rb * P : (rb + 1) * P],
                        rhs=attn2[:, h, j_ * QS : (j_ + 1) * QS],
                        start=(h == 0),
                        stop=(h == H - 1),
                    )
            o_t = opool.tile([P, RB, QS], F32, tag="o")
            for rb in range(RB):
                nc.vector.tensor_scalar_add(
                    out=o_t[:, rb, :], in0=ps[:, rb, :],
                    scalar1=bo_eff[:, rb : rb + 1],
                )
            nc.sync.dma_start(
                out=out_r[:, :, j_ * QS : (j_ + 1) * QS], in_=o_t
            )

        def sch_exp(e_sb, sp):
            # Schraudolph bf16 exp on the DVE: one fused mult+add into
            # int16, bitcast to bf16 (~1.5% rms err)
            nc.vector.tensor_scalar(
                out=e_sb.bitcast(mybir.dt.int16),
                in0=sp,
                scalar1=SCH_A,
                scalar2=SCH_B,
                op0=OP.mult,
                op1=OP.add,
            )

        def pair(j, hp, fused):
            prev = pend_norm[0]
            ou_ref = [None]
            rbc_ref = [None]
            avp = (
                avpool.tile([P, QS], F32, tag="av", name="avp0"),
                avpool.tile([P, QS], F32, tag="av", name="avp1"),
            )
            for c in range(NKC):
                if fused:
                    jk = c // 4
                    if c % 4 == 0:
                        emit_kproj(0, jk, "act")
                    elif c % 4 == 2:
                        emit_kproj(1, jk, "dve")
                    emit_vproj(c)
                sp = spool.tile([P, 2, QS], F32, tag="sp", name="sp")
                nc.tensor.matmul(
                    out=sp[:, 0, :],
                    lhsT=k2[0:D, hp, c * KC : (c + 1) * KC],
                    rhs=q2[0:D, hp, j * QS : (j + 1) * QS],
                    start=True, stop=True,
                )
                nc.tensor.matmul(
                    out=sp[:, 1, :],
                    lhsT=k2[D:P, hp, c * KC : (c + 1) * KC],
                    rhs=q2[D:P, hp, j * QS : (j + 1) * QS],
                    start=True, stop=True,
                )
                e_sb = epool.tile([P, 2, QS], BF16, tag="e")
                if fused:
                    if c % 3 == 2:
                        sch_exp(e_sb, sp)
                    else:
                        nc.scalar.activation(out=e_sb, in_=sp, func=AF.Exp)
                elif c % 2 == 0:
                    sch_exp(e_sb, sp)
                else:
                    nc.scalar.activation(out=e_sb, in_=sp, func=AF.Exp)
                # deferred normalize of the previous pair.  ALL reads of
                # pavp[u] must be emitted at c == 1+u (before the new AV
                # matmul to that PSUM bank is emitted).
                if prev is not None:
                    pj, php, pavp = prev
                    if c == 1:
                        ou = oupool.tile([D, 2, QS], F32, tag="ou")
                        ou_ref[0] = ou
                        den = oupool.tile([1, 2, QS], F32, tag="den")
                        rbc_ref[0] = [den, None]
                        nc.scalar.activation(
                            out=ou[:, 0, :], in_=pavp[0][0:D, :],
                            func=AF.Identity,
                        )
                        nc.vector.tensor_copy(
                            out=den[:, 0, :], in_=pavp[0][D : D + 1, :]
                        )
                    elif c == 2:
                        nc.vector.tensor_copy(
                            out=ou_ref[0][:, 1, :], in_=pavp[1][0:D, :]
                        )
                        nc.vector.tensor_copy(
                            out=rbc_ref[0][0][:, 1, :], in_=pavp[1][D : D + 1, :]
                        )
                    elif c == 4:
                        r0 = oupool.tile([1, 2, QS], F32, tag="r0")
                        nc.vector.reciprocal_approx_fast(
                            out=r0, in_=rbc_ref[0][0]
                        )
                        rbc_ref[0][0] = r0
                    elif c == 6:
                        rbc = rpool.tile([D, 2, QS], F32, tag="rbc")
                        rbc_ref[0][1] = rbc
                        for u in range(2):
                            nc.gpsimd.partition_broadcast(
                                rbc[:, u, :], rbc_ref[0][0][0:1, u, :]
                            )
                    elif c == 9:
                        nc.vector.tensor_tensor(
                            out=attn2[:, 2 * php : 2 * php + 2,
                                      pj * QS : (pj + 1) * QS],
                            in0=ou_ref[0],
                            in1=rbc_ref[0][1],
                            op=OP.mult,
                        )
                    elif c == 12 and php == 1:
                        outproj(pj)
                if pend_av[0] is not None:
                    emit_av(pend_av[0])
                pend_av[0] = (e_sb, c, avp, hp)
            pend_norm[0] = (j, hp, avp)

        first = True
        for j in range(NQS):
            for hp in range(HP):
                pair(j, hp, fused=first)
                first = False
        emit_av(pend_av[0])
        pend_av[0] = None

        # flush the last pair's normalize + outproj inline
        pj, php, pavp = pend_norm[0]
        ou = oupool.tile([D, 2, QS], F32, tag="ou")
        den = oupool.tile([1, 2, QS], F32, tag="den")
        nc.scalar.activation(out=ou[:, 0, :], in_=pavp[0][0:D, :],
                             func=AF.Identity)
        nc.vector.tensor_copy(out=den[:, 0, :], in_=pavp[0][D : D + 1, :])
        nc.vector.tensor_copy(out=ou[:, 1, :], in_=pavp[1][0:D, :])
        nc.vector.tensor_copy(out=den[:, 1, :], in_=pavp[1][D : D + 1, :])
        r0 = oupool.tile([1, 2, QS], F32, tag="r0")
        nc.vector.reciprocal_approx_fast(out=r0, in_=den)
        rbc = rpool.tile([D, 2, QS], F32, tag="rbc")
        for u in range(2):
            nc.gpsimd.partition_broadcast(rbc[:, u, :], r0[0:1, u, :])
        nc.vector.tensor_tensor(
            out=attn2[:, 2 * php : 2 * php + 2, pj * QS : (pj + 1) * QS],
            in0=ou, in1=rbc, op=OP.mult,
        )
        outproj(pj)


def build():
    nc = bacc.Bacc(
        "TRN2", target_bir_lowering=False, debug=False, num_devices=NCORES
    )
    x_mine = nc.dram_tensor("x_mine", [C, NH], XDT, kind="ExternalInput").ap()
    x_other = nc.dram_tensor("x_other", [C, NH], XDT, kind="ExternalInput").ap()
    x_rest = nc.dram_tensor(
        "x_rest", [3, 4, P, CT, N // 4], F8, kind="ExternalInput"
    ).ap()
    w_qkvT = nc.dram_tensor("w_qkvT", [C, 3 * C], XDT, kind="ExternalInput").ap()
    w_oT2 = nc.dram_tensor("w_oT2", [D, H, C], XDT, kind="ExternalInput").ap()
    bn_w = nc.dram_tensor("bn_w", [P, CT, 1], F32, kind="ExternalInput").ap()
    bn_b = nc.dram_tensor("bn_b", [P, CT, 1], F32, kind="ExternalInput").ap()
    b_out = nc.dram_tensor("b_out", [P, RB, 1], F32, kind="ExternalInput").ap()
    out = nc.dram_tensor("out", [C, NH], F32, kind="ExternalOutput").ap()
    with tile.TileContext(nc) as tc:
        _body(tc, x_mine, x_other, x_rest, w_qkvT, w_oT2, bn_w, bn_b, b_out, out)
    nc.compile()
    return nc


_nc_cache = None


def make_in_maps(x, bn_weight, bn_bias, w_qkv, w_out, b_out):
    import ml_dtypes

    x = np.ascontiguousarray(np.asarray(x, dtype=np.float32))
    x_bf = x.astype(ml_dtypes.bfloat16)
    x_f8 = x.astype(ml_dtypes.float8_e4m3fn)
    wqT = np.asarray(w_qkv, dtype=np.float32).T.copy()
    wqT[:, 0:C] *= SCALE  # fold d^-0.5 into the q columns
    wqT = wqT.astype(ml_dtypes.bfloat16)
    # w_out^T reorganized as [d, h, o] for the 4-matmul out-projection
    woT2 = np.ascontiguousarray(
        np.asarray(w_out, dtype=np.float32).T.reshape(H, D, C).transpose(1, 0, 2)
    ).astype(ml_dtypes.bfloat16)

    def vec_layout(v):
        v = np.asarray(v, dtype=np.float32)
        return np.ascontiguousarray(v.reshape(CT, P).T.reshape(P, CT, 1))

    bnw = vec_layout(bn_weight)
    bnb = vec_layout(bn_bias)
    bo = vec_layout(b_out)
    in_maps = []
    # x_rest layout [3, nchunk, P, CT, 1024]: contiguous per DMA chunk so the
    # stats-stream DMAs are pure sequential reads (c = ct*P + p)
    xr_all = x_f8.reshape(B, CT, P, 4, N // 4).transpose(0, 3, 2, 1, 4)
    for core in range(NCORES):
        bi, half = divmod(core, 2)
        mine = np.ascontiguousarray(x_bf[bi][:, half * NH : (half + 1) * NH])
        other = np.ascontiguousarray(x_bf[bi][:, (1 - half) * NH : (2 - half) * NH])
        rest = np.ascontiguousarray(xr_all[[b for b in range(B) if b != bi]])
        in_maps.append(
            {
                "x_mine": mine,
                "x_other": other,
                "x_rest": rest,
                "w_qkvT": wqT,
                "w_oT2": woT2,
                "bn_w": bnw,
                "bn_b": bnb,
                "b_out": bo,
            }
        )
    return in_maps


def assemble(results):
    outp = np.empty((B, C, N), np.float32)
    for core in range(NCORES):
        bi, half = divmod(core, 2)
        outp[bi][:, half * NH : (half + 1) * NH] = results[core]["out"]
    return outp


def kernel(x, bn_weight, bn_bias, w_qkv, w_out, b_out):
    global _nc_cache
    if _nc_cache is None:
        _nc_cache = build()
    in_maps = make_in_maps(x, bn_weight, bn_bias, w_qkv, w_out, b_out)
    res = run_bass_kernel_spmd(_nc_cache, in_maps, list(range(NCORES)))
    return assemble(res.results)


if __name__ == "__main__":
    rng = np.random.default_rng(0)
    x = rng.standard_normal((B, C, N), dtype=np.float32)
    w_qkv = rng.standard_normal((3 * C, C), dtype=np.float32) * C**-0.5
    w_out = rng.standard_normal((C, C), dtype=np.float32) * C**-0.5
    y = kernel(
        x,
        np.ones(C, np.float32),
        np.zeros(C, np.float32),
        w_qkv,
        w_out,
        np.zeros(C, np.float32),
    )
    print(y.shape, np.abs(y).max())


# revision 14
# speedup vs baseline: 1.3398x; 1.3398x over previous
"""TRN2 Bass/Tile kernel: BatchNorm1d + 4-head self-attention + out-projection.

Reference computation (b=4, c=256, n=4096, heads=4, d=64):
    xn   = BN(x)  (training-mode stats over batch+length)
    qkv  = w_qkv @ xn ;  q,k,v  (q scaled by d^-0.5)
    out  = softmax(q^T k) @ v^T  per (batch, head)
    y    = w_out @ out + b_out

Sharding over 8 NeuronCores: core i handles (batch i//2, query-half i%2).
Keys/values are processed in the core-local order [mine, other] (softmax and
attention are invariant to key permutation).

Design notes (v2 rewrite over the AllReduce-free baseline):
  - NO cross-core collective: every core receives the other 3 batches
    (fp8 for stats only) and computes the EXACT global BN statistics
    locally (own batch via DVE bn_stats, part of the rest via ACT
    Copy/Square accum_out sums).
  - BN scale folded into the QKV weights; shift becomes per-channel
    biases.  k-projection bias dropped (per-query score shift, softmax
    cancels it).  v-projection bias folded into the OUTPUT bias:
    W_out @ (W_v @ shift) is a per-output-channel constant, computed
    with tiny PE matmuls at startup -> zero steady-state cost.
  - HEAD-PAIR layout: q/k stored [128, hp, n] with head 2hp on
    partitions 0:64 and head 2hp+1 on 64:128 (no zero padding).  The
    scores for both heads of a pair run as TWO CONCURRENT K=64
    row-tiled matmuls (tile_position (0,0)/(64,0)) - measured 1.88x
    the serial K=128 rate on HW.
  - exp split 50/50: even key-chunks on ACT (table exp), odd chunks on
    the DVE as a Schraudolph bf16 bit-trick (fused mult+add to int16).
  - AV: lhsT = vT-block [128key, 65] bf16 (64 v channels + ones column
    -> softmax denominator for free); attn kept per-head at partitions
    0:64 (attn2 [64, h, n]); out-projection runs as 4 accumulating
    K=64 matmuls per 128-channel block (streaming time only depends on
    the moving size, so this costs ~nothing vs K=128).
  - Normalization (deferred into the next pair): ACT/DVE copy the AV
    PSUM to SBUF, DVE reciprocal_approx_fast in place, gpsimd
    partition_broadcast, one DVE multiply for both heads.
  - PSUM: spool 2x[128,2,512] (scores pairs, outproj, startup QKV) +
    avpool 4x[128,512] (AV accumulators + fused-phase k/v psum) = 8
    banks exactly.
  - k/v projections are fused INTO the first attention pair (produced
    just-in-time, one key-chunk ahead), so ACT/DVE exp work starts
    ~10us earlier; a short burst of keep-warm PE matmuls sequenced
    right before the stats-combine keeps the HAM clock at 8/8 when the
    real matmuls arrive.
"""

import numpy as np

import concourse.bacc as bacc
import concourse.tile as tile
from concourse import mybir
from concourse.bass_utils import run_bass_kernel_spmd

B, C, N = 4, 256, 4096
H, D = 4, 64
P = 128
CT = C // P            # 2 channel tiles of 128
RB = 2                 # row blocks for q/k rows (256 = 2*128)
HP = 2                 # head pairs
NH = N // 2            # 2048 queries per core
QS = 512               # query subtile (1 PSUM bank of fp32)
NQS = NH // QS         # 4
KC = 128               # key chunk (matmul stationary width)
NKC = N // KC          # 32
EPS = 1e-5
SCALE = D ** -0.5
F32 = mybir.dt.float32
BF16 = mybir.dt.bfloat16
XDT = BF16
F8 = mybir.dt.float8e4
NCORES = 8
WARM = 22          # keep-warm PE matmuls issued right before stats-combine
SCH_A = 184.6650244    # 2^7 / ln 2
SCH_B = 16250.65       # 127*128 - c_opt (half-way rounding compensation)


def _body(tc, x_mine, x_other, x_rest, w_qkvT, w_oT2, bn_w, bn_b, b_out, out):
    from contextlib import ExitStack

    nc = tc.nc
    AF = mybir.ActivationFunctionType
    OP = mybir.AluOpType

    with ExitStack() as ctx:
        big = ctx.enter_context(tc.tile_pool(name="big", bufs=1))
        small = ctx.enter_context(tc.tile_pool(name="small", bufs=1))
        epool = ctx.enter_context(tc.tile_pool(name="epool", bufs=4))
        oupool = ctx.enter_context(tc.tile_pool(name="oupool", bufs=2))
        rpool = ctx.enter_context(tc.tile_pool(name="rpool", bufs=2))
        opool = ctx.enter_context(tc.tile_pool(name="opool", bufs=2))
        spool = ctx.enter_context(tc.tile_pool(name="spool", bufs=3, space="PSUM"))
        avpool = ctx.enter_context(tc.tile_pool(name="avpool", bufs=2, space="PSUM"))

        # ---- loads: x_mine first (BN stats critical path) ---------------
        xn_sb = big.tile([P, CT, N], XDT, tag="xn")  # RAW x, key order [mine|other]
        xm_r = x_mine.rearrange("(ct p) n -> p ct n", p=P)
        for ct in range(CT):
            for half in range(2):
                nc.sync.dma_start(
                    out=xn_sb[:, ct, half * (NH // 2) : (half + 1) * (NH // 2)],
                    in_=xm_r[:, ct, half * (NH // 2) : (half + 1) * (NH // 2)],
                )
        nc.sync.dma_start(
            out=xn_sb[:, :, NH:N], in_=x_other.rearrange("(ct p) n -> p ct n", p=P)
        )
        wq_sb = big.tile([P, CT, 3 * C], XDT)
        nc.sync.dma_start(
            out=wq_sb, in_=w_qkvT.rearrange("(ct p) o -> p ct o", p=P)
        )
        wo2_sb = big.tile([D, H, C], XDT)   # w_out^T as [d, h, o]
        nc.sync.dma_start(out=wo2_sb, in_=w_oT2)
        bnw_sb = small.tile([P, CT, 1], F32)
        nc.sync.dma_start(out=bnw_sb, in_=bn_w)
        bnb_sb = small.tile([P, CT, 1], F32)
        nc.sync.dma_start(out=bnb_sb, in_=bn_b)
        bo_sb = small.tile([P, RB, 1], F32)
        nc.sync.dma_start(out=bo_sb, in_=b_out)

        # ---- BN stats: EXACT global stats computed locally --------------
        NRC = 4            # x_rest DMA chunks per batch (1024 fp8 cols)
        RCW = N // NRC
        SG = N // 512      # own-batch 512-col stat groups per ct
        NCH = 3 * NRC
        ACT_CH = (1, 3, 5, 8, 11)   # chunks reduced on ACT via accum sums
        NACT = len(ACT_CH)
        RG = RCW // 512
        NDVE_R = (NCH - NACT) * RG
        NREC = SG + NDVE_R         # bn_stats records per ct
        NS = N + NDVE_R * 512      # samples covered by bn_stats records
        NT = B * N                 # total samples per channel
        stat6 = small.tile([P, CT, NREC, 6], F32)
        for ct in range(CT):
            xm = xn_sb[:, ct, :].rearrange("p (s f) -> p s f", f=512)
            for s in range(SG):
                nc.vector.bn_stats(out=stat6[:, ct, s, :], in_=xm[:, s, :])
        stg = ctx.enter_context(tc.tile_pool(name="stg", bufs=4))
        trash = small.tile([P, RCW], BF16)
        acc_x = small.tile([P, CT, NACT], F32)
        acc_x2 = small.tile([P, CT, NACT], F32)
        ci_dve = 0
        ci_act = 0
        for rb_ in range(3):
            for chunk in range(NRC):
                st = stg.tile([P, CT, RCW], F8, tag="stg")
                nc.sync.dma_start(out=st, in_=x_rest[rb_, chunk])
                if rb_ * NRC + chunk in ACT_CH:
                    # ACT computes plain sums: Sum(x) via Copy-accumulate,
                    # Sum(x^2) via Square-accumulate
                    for ct in range(CT):
                        nc.scalar.activation(
                            out=trash, in_=st[:, ct, :], func=AF.Copy,
                            accum_out=acc_x[:, ct, ci_act : ci_act + 1],
                        )
                        nc.scalar.activation(
                            out=trash, in_=st[:, ct, :], func=AF.Square,
                            accum_out=acc_x2[:, ct, ci_act : ci_act + 1],
                        )
                    ci_act += 1
                else:
                    for ct in range(CT):
                        xr = st[:, ct, :].rearrange("p (s f) -> p s f", f=512)
                        for s in range(RG):
                            nc.vector.bn_stats(
                                out=stat6[:, ct, SG + ci_dve * RG + s, :],
                                in_=xr[:, s, :],
                            )
                    ci_dve += 1

        # ---- PE keep-warm ------------------------------------------------
        # The HAM activity monitor runs the PE at half clock until it has
        # seen a ~3.4us busy window.  A dense burst of N=512 matmuls DATA-
        # GATED on stat6 (so the scheduler cannot hoist it) runs during the
        # stats-combine and warms the clock just in time for the QKV
        # projections.
        dum2 = small.tile([1, QS], BF16)
        nc.vector.memset(dum2, 1.0)
        nc.vector.tensor_copy(
            out=dum2[0:1, 0 : NREC * 6],
            in_=stat6[0:1, 0, :, :].rearrange("p a b -> p (a b)"),
        )
        for i in range(WARM):
            scrap = spool.tile([P, 2, QS], F32, tag="sp", name="scrap")
            nc.tensor.matmul(
                out=scrap[0:1, 0, :], lhsT=dum2[0:1, 0:1], rhs=dum2,
                start=True, stop=True,
            )

        mv = small.tile([P, CT, 2], F32)
        for ct in range(CT):
            nc.vector.bn_aggr(out=mv[:, ct, :], in_=stat6[:, ct])
        # combine: totals = bn_aggr subset (NS samples) + ACT sums
        sum_t = small.tile([P, CT, 1], F32)
        nc.vector.tensor_reduce(
            out=sum_t, in_=acc_x, axis=mybir.AxisListType.X,
            op=mybir.AluOpType.add,
        )
        sq_t = small.tile([P, CT, 1], F32)
        nc.vector.tensor_reduce(
            out=sq_t, in_=acc_x2, axis=mybir.AxisListType.X,
            op=mybir.AluOpType.add,
        )
        msq_s = small.tile([P, CT, 1], F32)
        nc.vector.tensor_mul(out=msq_s, in0=mv[:, :, 0:1], in1=mv[:, :, 0:1])
        e2_s = small.tile([P, CT, 1], F32)
        nc.vector.tensor_add(out=e2_s, in0=mv[:, :, 1:2], in1=msq_s)
        # sum_t += mean_s * NS ; sq_t += e2_s * NS
        tmp_s = small.tile([P, CT, 1], F32)
        nc.vector.tensor_scalar_mul(out=tmp_s, in0=mv[:, :, 0:1], scalar1=float(NS))
        nc.vector.tensor_add(out=sum_t, in0=sum_t, in1=tmp_s)
        nc.vector.tensor_scalar_mul(out=tmp_s, in0=e2_s, scalar1=float(NS))
        nc.vector.tensor_add(out=sq_t, in0=sq_t, in1=tmp_s)
        mvg = small.tile([P, CT, 2], F32)
        nc.vector.tensor_scalar_mul(
            out=mvg[:, :, 0:1], in0=sum_t, scalar1=1.0 / NT
        )
        nc.vector.tensor_scalar_mul(out=tmp_s, in0=sq_t, scalar1=1.0 / NT)
        nc.vector.tensor_mul(
            out=mvg[:, :, 1:2], in0=mvg[:, :, 0:1], in1=mvg[:, :, 0:1]
        )
        nc.vector.tensor_sub(out=mvg[:, :, 1:2], in0=tmp_s, in1=mvg[:, :, 1:2])
        mv = mvg

        eps_sb = small.tile([P, 1], F32)
        nc.vector.memset(eps_sb, EPS)

        # ---- global mean/var -> s = bn_w * rstd, shift = bn_b - mean*s --
        mean_g = mv[:, :, 0:1]
        var_g = mv[:, :, 1:2]
        sd = small.tile([P, CT, 1], F32)
        nc.scalar.activation(out=sd, in_=var_g, func=AF.Sqrt, bias=eps_sb)
        rstd = small.tile([P, CT, 1], F32)
        nc.vector.reciprocal(out=rstd, in_=sd)
        s_sb = small.tile([P, CT, 1], F32)
        nc.vector.tensor_mul(out=s_sb, in0=bnw_sb, in1=rstd)
        shift_sb = small.tile([P, CT, 1], F32)
        nc.vector.tensor_mul(out=shift_sb, in0=mean_g, in1=s_sb)
        nc.vector.tensor_sub(out=shift_sb, in0=bnb_sb, in1=shift_sb)
        shift_bf = small.tile([P, CT, 1], BF16)
        nc.vector.tensor_copy(out=shift_bf, in_=shift_sb)

        # ---- biases from the ORIGINAL weights ---------------------------
        # q bias qkb[:, rb] = W_q[rb] @ shift (k bias dropped: softmax
        # cancels a per-query score shift).
        # v bias per head as a [64,1] column: vb4[:, h] = W_v,h @ shift.
        # Output-bias correction: bo_eff = b_out + W_out @ vb  (the v bias
        # contributes attn-weight-sum * vb = vb after normalization).
        bias_ps = spool.tile([P, 2, QS], F32, tag="sp", name="bias")
        for rb in range(RB):
            for ct in range(CT):
                nc.tensor.matmul(
                    out=bias_ps[:, 0, rb : rb + 1],
                    lhsT=wq_sb[:, ct, rb * P : (rb + 1) * P],
                    rhs=shift_bf[:, ct],
                    start=(ct == 0),
                    stop=(ct == CT - 1),
                )
        for h in range(H):
            for ct in range(CT):
                nc.tensor.matmul(
                    out=bias_ps[0:D, 0, RB + h : RB + h + 1],
                    lhsT=wq_sb[:, ct, 2 * C + h * D : 2 * C + (h + 1) * D],
                    rhs=shift_bf[:, ct],
                    start=(ct == 0),
                    stop=(ct == CT - 1),
                )
        qkb_sb = small.tile([P, RB], F32)
        nc.vector.tensor_copy(out=qkb_sb, in_=bias_ps[:, 0, 0:RB])
        vb4 = small.tile([D, H], BF16)
        nc.vector.tensor_copy(out=vb4, in_=bias_ps[0:D, 0, RB : RB + H])
        # bo_eff = b_out + sum_h W_out[:, (h,:)] @ vb4[:, h]
        for rb in range(RB):
            for h in range(H):
                nc.tensor.matmul(
                    out=bias_ps[:, 1, rb : rb + 1],
                    lhsT=wo2_sb[:, h, rb * P : (rb + 1) * P],
                    rhs=vb4[:, h : h + 1],
                    start=(h == 0),
                    stop=(h == H - 1),
                )
        bo_eff = small.tile([P, RB], F32)
        nc.vector.tensor_add(
            out=bo_eff, in0=bias_ps[:, 1, 0:RB],
            in1=bo_sb.rearrange("p rb one -> p (rb one)"),
        )

        # fold diag(s) into the weights, q columns first so q matmuls can
        # start while k/v columns are still being scaled (d^-0.5 is folded
        # into the q columns host-side)
        for sec in range(3):
            for ct in range(CT):
                nc.vector.tensor_scalar_mul(
                    out=wq_sb[:, ct, sec * C : (sec + 1) * C],
                    in0=wq_sb[:, ct, sec * C : (sec + 1) * C],
                    scalar1=s_sb[:, ct],
                )

        # ---- attention state --------------------------------------------
        q2 = big.tile([P, HP, NH], BF16)
        k2 = big.tile([P, HP, N], BF16)
        vT = big.tile([P, NKC, H, D + 1], BF16)
        nc.gpsimd.memset(vT[:, :, :, D : D + 1], 1.0)
        attn2 = big.tile([D, H, NH], BF16)
        out_r = out.rearrange("(rb p) n -> p rb n", p=P)

        # ---- q projection (consumes RAW x; bias fused into copies) ------
        for j in range(NQS):
            ps = spool.tile([P, 2, QS], F32, tag="sp", name="qproj")
            for hp in range(HP):
                for ct in range(CT):
                    nc.tensor.matmul(
                        out=ps[:, hp, :],
                        lhsT=wq_sb[:, ct, hp * P : (hp + 1) * P],
                        rhs=xn_sb[:, ct, j * QS : (j + 1) * QS],
                        start=(ct == 0),
                        stop=(ct == CT - 1),
                    )
            nc.vector.tensor_scalar_add(
                out=q2[:, 0, j * QS : (j + 1) * QS], in0=ps[:, 0, :],
                scalar1=qkb_sb[:, 0:1],
            )
            nc.scalar.activation(
                out=q2[:, 1, j * QS : (j + 1) * QS], in_=ps[:, 1, :],
                func=AF.Identity, bias=qkb_sb[:, 1:2],
            )

        # ---- k/v production (fused into the first pair's chunk loop) ---
        def emit_kproj(ps_slot, hp, jk, eng):
            for ct in range(CT):
                nc.tensor.matmul(
                    out=ps_slot,
                    lhsT=wq_sb[:, ct, C + hp * P : C + (hp + 1) * P],
                    rhs=xn_sb[:, ct, jk * QS : (jk + 1) * QS],
                    start=(ct == 0),
                    stop=(ct == CT - 1),
                )
            dst = k2[:, hp, jk * QS : (jk + 1) * QS]
            if eng == "act":
                nc.scalar.activation(out=dst, in_=ps_slot, func=AF.Identity)
            else:
                nc.vector.tensor_copy(out=dst, in_=ps_slot)

        def emit_vproj(ps_slot, nb):
            for ct in range(CT):
                nc.tensor.matmul(
                    out=ps_slot[:, 0:C],
                    lhsT=xn_sb[:, ct, nb * KC : (nb + 1) * KC],
                    rhs=wq_sb[:, ct, 2 * C : 3 * C],
                    start=(ct == 0),
                    stop=(ct == CT - 1),
                )
            nc.vector.tensor_copy(
                out=vT[:, nb, :, 0:D],
                in_=ps_slot[:, 0:C].rearrange("p (h d) -> p h d", d=D),
            )

        # ---- attention ---------------------------------------------------
        # Normalize/outproj of pair X are deferred into pair X+1 (hooks at
        # fixed chunk indices) so they never stall the exp engines.  The AV
        # matmuls run one chunk behind the scores (pend_av), carried across
        # pair boundaries.
        pend_av = [None]
        pend_norm = [None]

        def emit_av(p):
            e_, c_, avp_, hp_ = p
            for u in range(2):
                nc.tensor.matmul(
                    out=avp_[u][0 : D + 1, :],
                    lhsT=vT[:, c_, 2 * hp_ + u, :],
                    rhs=e_[:, u, :],
                    start=(c_ == 0),
                    stop=(c_ == NKC - 1),
                )

        def outproj(j_):
            ps = spool.tile([P, 2, QS], F32, tag="sp", name="oproj")
            for rb in range(RB):
                for h in range(H):
                    nc.tensor.matmul(
                        out=ps[:, rb, :],
                        lhsT=wo2_sb[:, h, rb * P : (rb + 1) * P],
                        rhs=attn2[:, h, j_ * QS : (j_ + 1) * QS],
                        start=(h == 0),
                        stop=(h == H - 1),
                    )
            o_t = opool.tile([P, RB, QS], F32, tag="o")
            for rb in range(RB):
                nc.vector.tensor_scalar_add(
                    out=o_t[:, rb, :], in0=ps[:, rb, :],
                    scalar1=bo_eff[:, rb : rb + 1],
                )
            nc.sync.dma_start(
                out=out_r[:, :, j_ * QS : (j_ + 1) * QS], in_=o_t
            )

        def sch_exp(e_sb, sp):
            # Schraudolph bf16 exp on the DVE: one fused mult+add into
            # int16, bitcast to bf16 (~1.5% rms err)
            nc.vector.tensor_scalar(
                out=e_sb.bitcast(mybir.dt.int16),
                in0=sp,
                scalar1=SCH_A,
                scalar2=SCH_B,
                op0=OP.mult,
                op1=OP.add,
            )

        def pair(j, hp, fused):
            prev = pend_norm[0]
            ou_ref = [None]
            rbc_ref = [None]
            avp = (
                avpool.tile([P, QS], F32, tag="av", name="avp0"),
                avpool.tile([P, QS], F32, tag="av", name="avp1"),
            )
            vt_ref = [None]
            for c in range(NKC):
                if fused:
                    jk = c // 4
                    if c % 4 == 0:
                        kt = spool.tile([P, 2, QS], F32, tag="sp", name="kproj")
                        emit_kproj(kt[:, 0, :], 0, jk, "act")
                        emit_kproj(kt[:, 1, :], 1, jk, "dve")
                    if c % 2 == 0:
                        vt_ref[0] = spool.tile(
                            [P, 2, QS], F32, tag="sp", name="vproj"
                        )
                    emit_vproj(vt_ref[0][:, c % 2, :], c)
                sp = spool.tile([P, 2, QS], F32, tag="sp", name="sp")
                nc.tensor.matmul(
                    out=sp[:, 0, :],
                    lhsT=k2[0:D, hp, c * KC : (c + 1) * KC],
                    rhs=q2[0:D, hp, j * QS : (j + 1) * QS],
                    start=True, stop=True,
                )
                nc.tensor.matmul(
                    out=sp[:, 1, :],
                    lhsT=k2[D:P, hp, c * KC : (c + 1) * KC],
                    rhs=q2[D:P, hp, j * QS : (j + 1) * QS],
                    start=True, stop=True,
                )
                e_sb = epool.tile([P, 2, QS], BF16, tag="e")
                if fused:
                    if c % 3 == 2:
                        sch_exp(e_sb, sp)
                    else:
                        nc.scalar.activation(out=e_sb, in_=sp, func=AF.Exp)
                elif c % 2 == 0:
                    sch_exp(e_sb, sp)
                else:
                    nc.scalar.activation(out=e_sb, in_=sp, func=AF.Exp)
                # deferred normalize of the previous pair.  ALL reads of
                # pavp[u] must be emitted at c == 1+u (before the new AV
                # matmul to that PSUM bank is emitted).
                if prev is not None:
                    pj, php, pavp = prev
                    if c == 1:
                        ou = oupool.tile([D, 2, QS], F32, tag="ou")
                        ou_ref[0] = ou
                        den = oupool.tile([1, 2, QS], F32, tag="den")
                        rbc_ref[0] = [den, None]
                        nc.scalar.activation(
                            out=ou[:, 0, :], in_=pavp[0][0:D, :],
                            func=AF.Identity,
                        )
                        nc.vector.tensor_copy(
                            out=den[:, 0, :], in_=pavp[0][D : D + 1, :]
                        )
                    elif c == 2:
                        nc.vector.tensor_copy(
                            out=ou_ref[0][:, 1, :], in_=pavp[1][0:D, :]
                        )
                        nc.vector.tensor_copy(
                            out=rbc_ref[0][0][:, 1, :], in_=pavp[1][D : D + 1, :]
                        )
                    elif c == 4:
                        r0 = oupool.tile([1, 2, QS], F32, tag="r0")
                        nc.vector.reciprocal_approx_fast(
                            out=r0, in_=rbc_ref[0][0]
                        )
                        rbc_ref[0][0] = r0
                    elif c == 6:
                        rbc = rpool.tile([D, 2, QS], F32, tag="rbc")
                        rbc_ref[0][1] = rbc
                        for u in range(2):
                            nc.gpsimd.partition_broadcast(
                                rbc[:, u, :], rbc_ref[0][0][0:1, u, :]
                            )
                    elif c == 9:
                        nc.vector.tensor_tensor(
                            out=attn2[:, 2 * php : 2 * php + 2,
                                      pj * QS : (pj + 1) * QS],
                            in0=ou_ref[0],
                            in1=rbc_ref[0][1],
                            op=OP.mult,
                        )
                    elif c == 12 and php == 1:
                        outproj(pj)
                if pend_av[0] is not None:
                    emit_av(pend_av[0])
                pend_av[0] = (e_sb, c, avp, hp)
            pend_norm[0] = (j, hp, avp)

        first = True
        for j in range(NQS):
            for hp in range(HP):
                pair(j, hp, fused=first)
                first = False
        emit_av(pend_av[0])
        pend_av[0] = None

        # flush the last pair's normalize + outproj inline
        pj, php, pavp = pend_norm[0]
        ou = oupool.tile([D, 2, QS], F32, tag="ou")
        den = oupool.tile([1, 2, QS], F32, tag="den")
        nc.scalar.activation(out=ou[:, 0, :], in_=pavp[0][0:D, :],
                             func=AF.Identity)
        nc.vector.tensor_copy(out=den[:, 0, :], in_=pavp[0][D : D + 1, :])
        nc.vector.tensor_copy(out=ou[:, 1, :], in_=pavp[1][0:D, :])
        nc.vector.tensor_copy(out=den[:, 1, :], in_=pavp[1][D : D + 1, :])
        r0 = oupool.tile([1, 2, QS], F32, tag="r0")
        nc.vector.reciprocal_approx_fast(out=r0, in_=den)
        rbc = rpool.tile([D, 2, QS], F32, tag="rbc")
        for u in range(2):
            nc.gpsimd.partition_broadcast(rbc[:, u, :], r0[0:1, u, :])
        nc.vector.tensor_tensor(
            out=attn2[:, 2 * php : 2 * php + 2, pj * QS : (pj + 1) * QS],
            in0=ou, in1=rbc, op=OP.mult,
        )
        outproj(pj)


def build():
    nc = bacc.Bacc(
        "TRN2", target_bir_lowering=False, debug=False, num_devices=NCORES
    )
    x_mine = nc.dram_tensor("x_mine", [C, NH], XDT, kind="ExternalInput").ap()
    x_other = nc.dram_tensor("x_other", [C, NH], XDT, kind="ExternalInput").ap()
    x_rest = nc.dram_tensor(
        "x_rest", [3, 4, P, CT, N // 4], F8, kind="ExternalInput"
    ).ap()
    w_qkvT = nc.dram_tensor("w_qkvT", [C, 3 * C], XDT, kind="ExternalInput").ap()
    w_oT2 = nc.dram_tensor("w_oT2", [D, H, C], XDT, kind="ExternalInput").ap()
    bn_w = nc.dram_tensor("bn_w", [P, CT, 1], F32, kind="ExternalInput").ap()
    bn_b = nc.dram_tensor("bn_b", [P, CT, 1], F32, kind="ExternalInput").ap()
    b_out = nc.dram_tensor("b_out", [P, RB, 1], F32, kind="ExternalInput").ap()
    out = nc.dram_tensor("out", [C, NH], F32, kind="ExternalOutput").ap()
    with tile.TileContext(nc) as tc:
        _body(tc, x_mine, x_other, x_rest, w_qkvT, w_oT2, bn_w, bn_b, b_out, out)
    nc.compile()
    return nc


_nc_cache = None


def make_in_maps(x, bn_weight, bn_bias, w_qkv, w_out, b_out):
    import ml_dtypes

    x = np.ascontiguousarray(np.asarray(x, dtype=np.float32))
    x_bf = x.astype(ml_dtypes.bfloat16)
    x_f8 = x.astype(ml_dtypes.float8_e4m3fn)
    wqT = np.asarray(w_qkv, dtype=np.float32).T.copy()
    wqT[:, 0:C] *= SCALE  # fold d^-0.5 into the q columns
    wqT = wqT.astype(ml_dtypes.bfloat16)
    # w_out^T reorganized as [d, h, o] for the 4-matmul out-projection
    woT2 = np.ascontiguousarray(
        np.asarray(w_out, dtype=np.float32).T.reshape(H, D, C).transpose(1, 0, 2)
    ).astype(ml_dtypes.bfloat16)

    def vec_layout(v):
        v = np.asarray(v, dtype=np.float32)
        return np.ascontiguousarray(v.reshape(CT, P).T.reshape(P, CT, 1))

    bnw = vec_layout(bn_weight)
    bnb = vec_layout(bn_bias)
    bo = vec_layout(b_out)
    in_maps = []
    # x_rest layout [3, nchunk, P, CT, 1024]: contiguous per DMA chunk so the
    # stats-stream DMAs are pure sequential reads (c = ct*P + p)
    xr_all = x_f8.reshape(B, CT, P, 4, N // 4).transpose(0, 3, 2, 1, 4)
    for core in range(NCORES):
        bi, half = divmod(core, 2)
        mine = np.ascontiguousarray(x_bf[bi][:, half * NH : (half + 1) * NH])
        other = np.ascontiguousarray(x_bf[bi][:, (1 - half) * NH : (2 - half) * NH])
        rest = np.ascontiguousarray(xr_all[[b for b in range(B) if b != bi]])
        in_maps.append(
            {
                "x_mine": mine,
                "x_other": other,
                "x_rest": rest,
                "w_qkvT": wqT,
                "w_oT2": woT2,
                "bn_w": bnw,
                "bn_b": bnb,
                "b_out": bo,
            }
        )
    return in_maps


def assemble(results):
    outp = np.empty((B, C, N), np.float32)
    for core in range(NCORES):
        bi, half = divmod(core, 2)
        outp[bi][:, half * NH : (half + 1) * NH] = results[core]["out"]
    return outp


def kernel(x, bn_weight, bn_bias, w_qkv, w_out, b_out):
    global _nc_cache
    if _nc_cache is None:
        _nc_cache = build()
    in_maps = make_in_maps(x, bn_weight, bn_bias, w_qkv, w_out, b_out)
    res = run_bass_kernel_spmd(_nc_cache, in_maps, list(range(NCORES)))
    return assemble(res.results)


if __name__ == "__main__":
    rng = np.random.default_rng(0)
    x = rng.standard_normal((B, C, N), dtype=np.float32)
    w_qkv = rng.standard_normal((3 * C, C), dtype=np.float32) * C**-0.5
    w_out = rng.standard_normal((C, C), dtype=np.float32) * C**-0.5
    y = kernel(
        x,
        np.ones(C, np.float32),
        np.zeros(C, np.float32),
        w_qkv,
        w_out,
        np.zeros(C, np.float32),
    )
    print(y.shape, np.abs(y).max())


# revision 15
# speedup vs baseline: 1.4091x; 1.0518x over previous
"""TRN2 Bass/Tile kernel: BatchNorm1d + 4-head self-attention + out-projection.

Reference computation (b=4, c=256, n=4096, heads=4, d=64):
    xn   = BN(x)  (training-mode stats over batch+length)
    qkv  = w_qkv @ xn ;  q,k,v  (q scaled by d^-0.5)
    out  = softmax(q^T k) @ v^T  per (batch, head)
    y    = w_out @ out + b_out

Sharding over 8 NeuronCores: core i handles (batch i//2, query-half i%2).
Keys/values are processed in the core-local order [mine, other] (softmax and
attention are invariant to key permutation).

Design notes (v2 rewrite over the AllReduce-free baseline):
  - NO cross-core collective: every core receives the other 3 batches
    (fp8 for stats only) and computes the EXACT global BN statistics
    locally (own batch via DVE bn_stats, part of the rest via ACT
    Copy/Square accum_out sums).
  - BN scale folded into the QKV weights; shift becomes per-channel
    biases.  k-projection bias dropped (per-query score shift, softmax
    cancels it).  v-projection bias folded into the OUTPUT bias:
    W_out @ (W_v @ shift) is a per-output-channel constant, computed
    with tiny PE matmuls at startup -> zero steady-state cost.
  - HEAD-PAIR layout: q/k stored [128, hp, n] with head 2hp on
    partitions 0:64 and head 2hp+1 on 64:128 (no zero padding).  The
    scores for both heads of a pair run as TWO CONCURRENT K=64
    row-tiled matmuls (tile_position (0,0)/(64,0)) - measured 1.88x
    the serial K=128 rate on HW.
  - exp split 50/50: even key-chunks on ACT (table exp), odd chunks on
    the DVE as a Schraudolph bf16 bit-trick (fused mult+add to int16).
  - AV: lhsT = vT-block [128key, 65] bf16 (64 v channels + ones column
    -> softmax denominator for free); attn kept per-head at partitions
    0:64 (attn2 [64, h, n]); out-projection runs as 4 accumulating
    K=64 matmuls per 128-channel block (streaming time only depends on
    the moving size, so this costs ~nothing vs K=128).
  - Normalization (deferred into the next pair): ACT/DVE copy the AV
    PSUM to SBUF, DVE reciprocal_approx_fast in place, gpsimd
    partition_broadcast, one DVE multiply for both heads.
  - PSUM: spool 2x[128,2,512] (scores pairs, outproj, startup QKV) +
    avpool 4x[128,512] (AV accumulators + fused-phase k/v psum) = 8
    banks exactly.
  - k/v projections are fused INTO the first attention pair (produced
    just-in-time, one key-chunk ahead), so ACT/DVE exp work starts
    ~10us earlier; a short burst of keep-warm PE matmuls sequenced
    right before the stats-combine keeps the HAM clock at 8/8 when the
    real matmuls arrive.
"""

import numpy as np

import concourse.bacc as bacc
import concourse.tile as tile
from concourse import mybir
from concourse.bass_utils import run_bass_kernel_spmd

B, C, N = 4, 256, 4096
H, D = 4, 64
P = 128
CT = C // P            # 2 channel tiles of 128
RB = 2                 # row blocks for q/k rows (256 = 2*128)
HP = 2                 # head pairs
NH = N // 2            # 2048 queries per core
QS = 512               # query subtile (1 PSUM bank of fp32)
NQS = NH // QS         # 4
KC = 128               # key chunk (matmul stationary width)
NKC = N // KC          # 32
EPS = 1e-5
SCALE = D ** -0.5
F32 = mybir.dt.float32
BF16 = mybir.dt.bfloat16
XDT = BF16
F8 = mybir.dt.float8e4
NCORES = 8
WARM = 22          # keep-warm PE matmuls issued right before stats-combine
SCH_A = 184.6650244    # 2^7 / ln 2
SCH_B = 16250.65       # 127*128 - c_opt (half-way rounding compensation)


def _body(tc, x_mine, x_other, x_rest, w_qkvT, w_oT2, bn_w, bn_b, b_out, out):
    from contextlib import ExitStack

    nc = tc.nc
    AF = mybir.ActivationFunctionType
    OP = mybir.AluOpType

    with ExitStack() as ctx:
        big = ctx.enter_context(tc.tile_pool(name="big", bufs=1))
        small = ctx.enter_context(tc.tile_pool(name="small", bufs=1))
        epool = ctx.enter_context(tc.tile_pool(name="epool", bufs=4))
        oupool = ctx.enter_context(tc.tile_pool(name="oupool", bufs=2))
        rpool = ctx.enter_context(tc.tile_pool(name="rpool", bufs=2))
        opool = ctx.enter_context(tc.tile_pool(name="opool", bufs=2))
        spool = ctx.enter_context(tc.tile_pool(name="spool", bufs=3, space="PSUM"))
        avpool = ctx.enter_context(tc.tile_pool(name="avpool", bufs=2, space="PSUM"))

        # ---- loads: x_mine first (BN stats critical path) ---------------
        xn_sb = big.tile([P, CT, N], XDT, tag="xn")  # RAW x, key order [mine|other]
        xm_r = x_mine.rearrange("(ct p) n -> p ct n", p=P)
        for ct in range(CT):
            for half in range(2):
                nc.sync.dma_start(
                    out=xn_sb[:, ct, half * (NH // 2) : (half + 1) * (NH // 2)],
                    in_=xm_r[:, ct, half * (NH // 2) : (half + 1) * (NH // 2)],
                )
        nc.sync.dma_start(
            out=xn_sb[:, :, NH:N], in_=x_other.rearrange("(ct p) n -> p ct n", p=P)
        )
        wq_sb = big.tile([P, CT, 3 * C], XDT)
        nc.sync.dma_start(
            out=wq_sb, in_=w_qkvT.rearrange("(ct p) o -> p ct o", p=P)
        )
        wo2_sb = big.tile([D, H, C], XDT)   # w_out^T as [d, h, o]
        nc.sync.dma_start(out=wo2_sb, in_=w_oT2)
        bnw_sb = small.tile([P, CT, 1], F32)
        nc.sync.dma_start(out=bnw_sb, in_=bn_w)
        bnb_sb = small.tile([P, CT, 1], F32)
        nc.sync.dma_start(out=bnb_sb, in_=bn_b)
        bo_sb = small.tile([P, RB, 1], F32)
        nc.sync.dma_start(out=bo_sb, in_=b_out)

        # ---- BN stats: EXACT global stats computed locally --------------
        NRC = 4            # x_rest DMA chunks per batch (1024 fp8 cols)
        RCW = N // NRC
        SG = N // 512      # own-batch 512-col stat groups per ct
        NCH = 3 * NRC
        ACT_CH = (1, 3, 5, 8, 11)   # chunks reduced on ACT via accum sums
        NACT = len(ACT_CH)
        RG = RCW // 512
        NDVE_R = (NCH - NACT) * RG
        NREC = SG + NDVE_R         # bn_stats records per ct
        NS = N + NDVE_R * 512      # samples covered by bn_stats records
        NT = B * N                 # total samples per channel
        stat6 = small.tile([P, CT, NREC, 6], F32)
        for ct in range(CT):
            xm = xn_sb[:, ct, :].rearrange("p (s f) -> p s f", f=512)
            for s in range(SG):
                nc.vector.bn_stats(out=stat6[:, ct, s, :], in_=xm[:, s, :])
        stg = ctx.enter_context(tc.tile_pool(name="stg", bufs=4))
        trash = small.tile([P, RCW], BF16)
        acc_x = small.tile([P, CT, NACT], F32)
        acc_x2 = small.tile([P, CT, NACT], F32)
        ci_dve = 0
        ci_act = 0
        for rb_ in range(3):
            for chunk in range(NRC):
                st = stg.tile([P, CT, RCW], F8, tag="stg")
                nc.sync.dma_start(out=st, in_=x_rest[rb_, chunk])
                if rb_ * NRC + chunk in ACT_CH:
                    # ACT computes plain sums: Sum(x) via Copy-accumulate,
                    # Sum(x^2) via Square-accumulate
                    for ct in range(CT):
                        nc.scalar.activation(
                            out=trash, in_=st[:, ct, :], func=AF.Copy,
                            accum_out=acc_x[:, ct, ci_act : ci_act + 1],
                        )
                        nc.scalar.activation(
                            out=trash, in_=st[:, ct, :], func=AF.Square,
                            accum_out=acc_x2[:, ct, ci_act : ci_act + 1],
                        )
                    ci_act += 1
                else:
                    for ct in range(CT):
                        xr = st[:, ct, :].rearrange("p (s f) -> p s f", f=512)
                        for s in range(RG):
                            nc.vector.bn_stats(
                                out=stat6[:, ct, SG + ci_dve * RG + s, :],
                                in_=xr[:, s, :],
                            )
                    ci_dve += 1

        # ---- PE keep-warm ------------------------------------------------
        # The HAM activity monitor runs the PE at half clock until it has
        # seen a ~3.4us busy window.  A dense burst of N=512 matmuls DATA-
        # GATED on stat6 (so the scheduler cannot hoist it) runs during the
        # stats-combine and warms the clock just in time for the QKV
        # projections.
        dum2 = small.tile([1, QS], BF16)
        nc.vector.memset(dum2, 1.0)
        nc.vector.tensor_copy(
            out=dum2[0:1, 0 : NREC * 6],
            in_=stat6[0:1, 0, :, :].rearrange("p a b -> p (a b)"),
        )
        for i in range(WARM):
            scrap = spool.tile([P, 2, QS], F32, tag="sp", name="scrap")
            nc.tensor.matmul(
                out=scrap[0:1, 0, :], lhsT=dum2[0:1, 0:1], rhs=dum2,
                start=True, stop=True,
            )

        mv = small.tile([P, CT, 2], F32)
        for ct in range(CT):
            nc.vector.bn_aggr(out=mv[:, ct, :], in_=stat6[:, ct])
        # combine: totals = bn_aggr subset (NS samples) + ACT sums
        sum_t = small.tile([P, CT, 1], F32)
        nc.vector.tensor_reduce(
            out=sum_t, in_=acc_x, axis=mybir.AxisListType.X,
            op=mybir.AluOpType.add,
        )
        sq_t = small.tile([P, CT, 1], F32)
        nc.vector.tensor_reduce(
            out=sq_t, in_=acc_x2, axis=mybir.AxisListType.X,
            op=mybir.AluOpType.add,
        )
        msq_s = small.tile([P, CT, 1], F32)
        nc.vector.tensor_mul(out=msq_s, in0=mv[:, :, 0:1], in1=mv[:, :, 0:1])
        e2_s = small.tile([P, CT, 1], F32)
        nc.vector.tensor_add(out=e2_s, in0=mv[:, :, 1:2], in1=msq_s)
        # sum_t += mean_s * NS ; sq_t += e2_s * NS
        tmp_s = small.tile([P, CT, 1], F32)
        nc.vector.tensor_scalar_mul(out=tmp_s, in0=mv[:, :, 0:1], scalar1=float(NS))
        nc.vector.tensor_add(out=sum_t, in0=sum_t, in1=tmp_s)
        nc.vector.tensor_scalar_mul(out=tmp_s, in0=e2_s, scalar1=float(NS))
        nc.vector.tensor_add(out=sq_t, in0=sq_t, in1=tmp_s)
        mvg = small.tile([P, CT, 2], F32)
        nc.vector.tensor_scalar_mul(
            out=mvg[:, :, 0:1], in0=sum_t, scalar1=1.0 / NT
        )
        nc.vector.tensor_scalar_mul(out=tmp_s, in0=sq_t, scalar1=1.0 / NT)
        nc.vector.tensor_mul(
            out=mvg[:, :, 1:2], in0=mvg[:, :, 0:1], in1=mvg[:, :, 0:1]
        )
        nc.vector.tensor_sub(out=mvg[:, :, 1:2], in0=tmp_s, in1=mvg[:, :, 1:2])
        mv = mvg

        eps_sb = small.tile([P, 1], F32)
        nc.vector.memset(eps_sb, EPS)

        # ---- global mean/var -> s = bn_w * rstd, shift = bn_b - mean*s --
        mean_g = mv[:, :, 0:1]
        var_g = mv[:, :, 1:2]
        sd = small.tile([P, CT, 1], F32)
        nc.scalar.activation(out=sd, in_=var_g, func=AF.Sqrt, bias=eps_sb)
        rstd = small.tile([P, CT, 1], F32)
        nc.vector.reciprocal(out=rstd, in_=sd)
        s_sb = small.tile([P, CT, 1], F32)
        nc.vector.tensor_mul(out=s_sb, in0=bnw_sb, in1=rstd)
        shift_sb = small.tile([P, CT, 1], F32)
        nc.vector.tensor_mul(out=shift_sb, in0=mean_g, in1=s_sb)
        nc.vector.tensor_sub(out=shift_sb, in0=bnb_sb, in1=shift_sb)
        shift_bf = small.tile([P, CT, 1], BF16)
        nc.vector.tensor_copy(out=shift_bf, in_=shift_sb)

        # ---- biases from the ORIGINAL weights ---------------------------
        # q bias qkb[:, rb] = W_q[rb] @ shift (k bias dropped: softmax
        # cancels a per-query score shift).
        # v bias per head as a [64,1] column: vb4[:, h] = W_v,h @ shift.
        # Output-bias correction: bo_eff = b_out + W_out @ vb  (the v bias
        # contributes attn-weight-sum * vb = vb after normalization).
        bias_ps = spool.tile([P, 2, QS], F32, tag="sp", name="bias")
        for rb in range(RB):
            for ct in range(CT):
                nc.tensor.matmul(
                    out=bias_ps[:, 0, rb : rb + 1],
                    lhsT=wq_sb[:, ct, rb * P : (rb + 1) * P],
                    rhs=shift_bf[:, ct],
                    start=(ct == 0),
                    stop=(ct == CT - 1),
                )
        for h in range(H):
            for ct in range(CT):
                nc.tensor.matmul(
                    out=bias_ps[0:D, 0, RB + h : RB + h + 1],
                    lhsT=wq_sb[:, ct, 2 * C + h * D : 2 * C + (h + 1) * D],
                    rhs=shift_bf[:, ct],
                    start=(ct == 0),
                    stop=(ct == CT - 1),
                )
        qkb_sb = small.tile([P, RB], F32)
        nc.vector.tensor_copy(out=qkb_sb, in_=bias_ps[:, 0, 0:RB])
        vb4 = small.tile([D, H], BF16)
        nc.vector.tensor_copy(out=vb4, in_=bias_ps[0:D, 0, RB : RB + H])
        # bo_eff = b_out + sum_h W_out[:, (h,:)] @ vb4[:, h]
        for rb in range(RB):
            for h in range(H):
                nc.tensor.matmul(
                    out=bias_ps[:, 1, rb : rb + 1],
                    lhsT=wo2_sb[:, h, rb * P : (rb + 1) * P],
                    rhs=vb4[:, h : h + 1],
                    start=(h == 0),
                    stop=(h == H - 1),
                )
        bo_eff = small.tile([P, RB], F32)
        nc.vector.tensor_add(
            out=bo_eff, in0=bias_ps[:, 1, 0:RB],
            in1=bo_sb.rearrange("p rb one -> p (rb one)"),
        )

        # fold diag(s) into the weights, q columns first so q matmuls can
        # start while k/v columns are still being scaled (d^-0.5 is folded
        # into the q columns host-side)
        for sec in range(3):
            for ct in range(CT):
                nc.vector.tensor_scalar_mul(
                    out=wq_sb[:, ct, sec * C : (sec + 1) * C],
                    in0=wq_sb[:, ct, sec * C : (sec + 1) * C],
                    scalar1=s_sb[:, ct],
                )

        # ---- attention state --------------------------------------------
        q2 = big.tile([P, HP, NH], BF16)
        k2 = big.tile([P, HP, N], BF16)
        vT = big.tile([P, NKC, H, D + 1], BF16)
        nc.gpsimd.memset(vT[:, :, :, D : D + 1], 1.0)
        attn2 = big.tile([D, H, NH], BF16)
        out_r = out.rearrange("(rb p) n -> p rb n", p=P)

        # ---- q projection (consumes RAW x; bias fused into copies) ------
        for j in range(NQS):
            ps = spool.tile([P, 2, QS], F32, tag="sp", name="qproj")
            for hp in range(HP):
                for ct in range(CT):
                    nc.tensor.matmul(
                        out=ps[:, hp, :],
                        lhsT=wq_sb[:, ct, hp * P : (hp + 1) * P],
                        rhs=xn_sb[:, ct, j * QS : (j + 1) * QS],
                        start=(ct == 0),
                        stop=(ct == CT - 1),
                    )
            nc.vector.tensor_scalar_add(
                out=q2[:, 0, j * QS : (j + 1) * QS], in0=ps[:, 0, :],
                scalar1=qkb_sb[:, 0:1],
            )
            nc.scalar.activation(
                out=q2[:, 1, j * QS : (j + 1) * QS], in_=ps[:, 1, :],
                func=AF.Identity, bias=qkb_sb[:, 1:2],
            )

        # ---- k/v production (fused into the first pair's chunk loop) ---
        def emit_kproj(ps_slot, hp, jk, eng):
            for ct in range(CT):
                nc.tensor.matmul(
                    out=ps_slot,
                    lhsT=wq_sb[:, ct, C + hp * P : C + (hp + 1) * P],
                    rhs=xn_sb[:, ct, jk * QS : (jk + 1) * QS],
                    start=(ct == 0),
                    stop=(ct == CT - 1),
                )
            dst = k2[:, hp, jk * QS : (jk + 1) * QS]
            if eng == "act":
                nc.scalar.activation(out=dst, in_=ps_slot, func=AF.Identity)
            else:
                nc.vector.tensor_copy(out=dst, in_=ps_slot)

        def emit_vproj(ps_slot, nb):
            for ct in range(CT):
                nc.tensor.matmul(
                    out=ps_slot[:, 0:C],
                    lhsT=xn_sb[:, ct, nb * KC : (nb + 1) * KC],
                    rhs=wq_sb[:, ct, 2 * C : 3 * C],
                    start=(ct == 0),
                    stop=(ct == CT - 1),
                )
            nc.vector.tensor_copy(
                out=vT[:, nb, :, 0:D],
                in_=ps_slot[:, 0:C].rearrange("p (h d) -> p h d", d=D),
            )

        # ---- attention ---------------------------------------------------
        # Normalize/outproj of pair X are deferred into pair X+1 (hooks at
        # fixed chunk indices) so they never stall the exp engines.  The AV
        # matmuls run one chunk behind the scores (pend_av), carried across
        # pair boundaries.
        pend_av = []          # queue of up to 2 pending AV chunk emissions
        pend_norm = [None]

        def emit_av(p):
            e_, c_, avp_, hp_ = p
            for u in range(2):
                nc.tensor.matmul(
                    out=avp_[u][0 : D + 1, :],
                    lhsT=vT[:, c_, 2 * hp_ + u, :],
                    rhs=e_[:, u, :],
                    start=(c_ == 0),
                    stop=(c_ == NKC - 1),
                )

        def outproj(j_):
            ps = spool.tile([P, 2, QS], F32, tag="sp", name="oproj")
            for rb in range(RB):
                for h in range(H):
                    nc.tensor.matmul(
                        out=ps[:, rb, :],
                        lhsT=wo2_sb[:, h, rb * P : (rb + 1) * P],
                        rhs=attn2[:, h, j_ * QS : (j_ + 1) * QS],
                        start=(h == 0),
                        stop=(h == H - 1),
                    )
            o_t = opool.tile([P, RB, QS], F32, tag="o")
            for rb in range(RB):
                nc.scalar.activation(
                    out=o_t[:, rb, :], in_=ps[:, rb, :],
                    func=AF.Identity, bias=bo_eff[:, rb : rb + 1],
                )
            nc.sync.dma_start(
                out=out_r[:, :, j_ * QS : (j_ + 1) * QS], in_=o_t
            )

        def sch_exp(e_sb, sp):
            # Schraudolph bf16 exp on the DVE: one fused mult+add into
            # int16, bitcast to bf16 (~1.5% rms err)
            nc.vector.tensor_scalar(
                out=e_sb.bitcast(mybir.dt.int16),
                in0=sp,
                scalar1=SCH_A,
                scalar2=SCH_B,
                op0=OP.mult,
                op1=OP.add,
            )

        def pair(j, hp, fused):
            prev = pend_norm[0]
            ou_ref = [None]
            rbc_ref = [None]
            avp = (
                avpool.tile([P, QS], F32, tag="av", name="avp0"),
                avpool.tile([P, QS], F32, tag="av", name="avp1"),
            )
            vt_ref = [None]
            for c in range(NKC):
                if fused:
                    jk = c // 4
                    if c % 4 == 0:
                        kt = spool.tile([P, 2, QS], F32, tag="sp", name="kproj")
                        emit_kproj(kt[:, 0, :], 0, jk, "act")
                        emit_kproj(kt[:, 1, :], 1, jk, "dve")
                    if c % 2 == 0:
                        vt_ref[0] = spool.tile(
                            [P, 2, QS], F32, tag="sp", name="vproj"
                        )
                    emit_vproj(vt_ref[0][:, c % 2, :], c)
                sp = spool.tile([P, 2, QS], F32, tag="sp", name="sp")
                nc.tensor.matmul(
                    out=sp[:, 0, :],
                    lhsT=k2[0:D, hp, c * KC : (c + 1) * KC],
                    rhs=q2[0:D, hp, j * QS : (j + 1) * QS],
                    start=True, stop=True,
                )
                nc.tensor.matmul(
                    out=sp[:, 1, :],
                    lhsT=k2[D:P, hp, c * KC : (c + 1) * KC],
                    rhs=q2[D:P, hp, j * QS : (j + 1) * QS],
                    start=True, stop=True,
                )
                e_sb = epool.tile([P, 2, QS], BF16, tag="e")
                if fused:
                    if c % 3 == 2:
                        sch_exp(e_sb, sp)
                    else:
                        nc.scalar.activation(out=e_sb, in_=sp, func=AF.Exp)
                elif c % 2 == 0 and c > 0:
                    sch_exp(e_sb, sp)
                else:
                    nc.scalar.activation(out=e_sb, in_=sp, func=AF.Exp)
                # deferred normalize of the previous pair.  ALL reads of
                # pavp[u] must be emitted at c == 1+u (before the new AV
                # matmul to that PSUM bank is emitted).
                if prev is not None:
                    pj, php, pavp = prev
                    if c == 2:
                        ou = oupool.tile([D, 2, QS], F32, tag="ou")
                        ou_ref[0] = ou
                        den = oupool.tile([1, 2, QS], F32, tag="den")
                        rbc_ref[0] = [den, None]
                        nc.scalar.activation(
                            out=ou[:, 0, :], in_=pavp[0][0:D, :],
                            func=AF.Identity,
                        )
                        nc.vector.tensor_copy(
                            out=den[:, 0, :], in_=pavp[0][D : D + 1, :]
                        )
                    elif c == 3:
                        nc.vector.tensor_copy(
                            out=ou_ref[0][:, 1, :], in_=pavp[1][0:D, :]
                        )
                        nc.vector.tensor_copy(
                            out=rbc_ref[0][0][:, 1, :], in_=pavp[1][D : D + 1, :]
                        )
                    elif c == 5:
                        r0 = oupool.tile([1, 2, QS], F32, tag="r0")
                        nc.vector.reciprocal_approx_fast(
                            out=r0, in_=rbc_ref[0][0]
                        )
                        rbc_ref[0][0] = r0
                    elif c == 7:
                        rbc = rpool.tile([D, 2, QS], F32, tag="rbc")
                        rbc_ref[0][1] = rbc
                        for u in range(2):
                            nc.gpsimd.partition_broadcast(
                                rbc[:, u, :], rbc_ref[0][0][0:1, u, :]
                            )
                    elif c == 10:
                        nc.vector.tensor_tensor(
                            out=attn2[:, 2 * php : 2 * php + 2,
                                      pj * QS : (pj + 1) * QS],
                            in0=ou_ref[0],
                            in1=rbc_ref[0][1],
                            op=OP.mult,
                        )
                    elif c == 13 and php == 1:
                        outproj(pj)
                if len(pend_av) >= 2:
                    emit_av(pend_av.pop(0))
                pend_av.append((e_sb, c, avp, hp))
            pend_norm[0] = (j, hp, avp)

        first = True
        for j in range(NQS):
            for hp in range(HP):
                pair(j, hp, fused=first)
                first = False
        while pend_av:
            emit_av(pend_av.pop(0))

        # flush the last pair's normalize + outproj inline
        pj, php, pavp = pend_norm[0]
        ou = oupool.tile([D, 2, QS], F32, tag="ou")
        den = oupool.tile([1, 2, QS], F32, tag="den")
        nc.scalar.activation(out=ou[:, 0, :], in_=pavp[0][0:D, :],
                             func=AF.Identity)
        nc.vector.tensor_copy(out=den[:, 0, :], in_=pavp[0][D : D + 1, :])
        nc.vector.tensor_copy(out=ou[:, 1, :], in_=pavp[1][0:D, :])
        nc.vector.tensor_copy(out=den[:, 1, :], in_=pavp[1][D : D + 1, :])
        r0 = oupool.tile([1, 2, QS], F32, tag="r0")
        nc.vector.reciprocal_approx_fast(out=r0, in_=den)
        rbc = rpool.tile([D, 2, QS], F32, tag="rbc")
        for u in range(2):
            nc.gpsimd.partition_broadcast(rbc[:, u, :], r0[0:1, u, :])
        nc.vector.tensor_tensor(
            out=attn2[:, 2 * php : 2 * php + 2, pj * QS : (pj + 1) * QS],
            in0=ou, in1=rbc, op=OP.mult,
        )
        outproj(pj)


def build():
    nc = bacc.Bacc(
        "TRN2", target_bir_lowering=False, debug=False, num_devices=NCORES
    )
    x_mine = nc.dram_tensor("x_mine", [C, NH], XDT, kind="ExternalInput").ap()
    x_other = nc.dram_tensor("x_other", [C, NH], XDT, kind="ExternalInput").ap()
    x_rest = nc.dram_tensor(
        "x_rest", [3, 4, P, CT, N // 4], F8, kind="ExternalInput"
    ).ap()
    w_qkvT = nc.dram_tensor("w_qkvT", [C, 3 * C], XDT, kind="ExternalInput").ap()
    w_oT2 = nc.dram_tensor("w_oT2", [D, H, C], XDT, kind="ExternalInput").ap()
    bn_w = nc.dram_tensor("bn_w", [P, CT, 1], F32, kind="ExternalInput").ap()
    bn_b = nc.dram_tensor("bn_b", [P, CT, 1], F32, kind="ExternalInput").ap()
    b_out = nc.dram_tensor("b_out", [P, RB, 1], F32, kind="ExternalInput").ap()
    out = nc.dram_tensor("out", [C, NH], F32, kind="ExternalOutput").ap()
    with tile.TileContext(nc) as tc:
        _body(tc, x_mine, x_other, x_rest, w_qkvT, w_oT2, bn_w, bn_b, b_out, out)
    nc.compile()
    return nc


_nc_cache = None


def make_in_maps(x, bn_weight, bn_bias, w_qkv, w_out, b_out):
    import ml_dtypes

    x = np.ascontiguousarray(np.asarray(x, dtype=np.float32))
    x_bf = x.astype(ml_dtypes.bfloat16)
    x_f8 = x.astype(ml_dtypes.float8_e4m3fn)
    wqT = np.asarray(w_qkv, dtype=np.float32).T.copy()
    wqT[:, 0:C] *= SCALE  # fold d^-0.5 into the q columns
    wqT = wqT.astype(ml_dtypes.bfloat16)
    # w_out^T reorganized as [d, h, o] for the 4-matmul out-projection
    woT2 = np.ascontiguousarray(
        np.asarray(w_out, dtype=np.float32).T.reshape(H, D, C).transpose(1, 0, 2)
    ).astype(ml_dtypes.bfloat16)

    def vec_layout(v):
        v = np.asarray(v, dtype=np.float32)
        return np.ascontiguousarray(v.reshape(CT, P).T.reshape(P, CT, 1))

    bnw = vec_layout(bn_weight)
    bnb = vec_layout(bn_bias)
    bo = vec_layout(b_out)
    in_maps = []
    # x_rest layout [3, nchunk, P, CT, 1024]: contiguous per DMA chunk so the
    # stats-stream DMAs are pure sequential reads (c = ct*P + p)
    xr_all = x_f8.reshape(B, CT, P, 4, N // 4).transpose(0, 3, 2, 1, 4)
    for core in range(NCORES):
        bi, half = divmod(core, 2)
        mine = np.ascontiguousarray(x_bf[bi][:, half * NH : (half + 1) * NH])
        other = np.ascontiguousarray(x_bf[bi][:, (1 - half) * NH : (2 - half) * NH])
        rest = np.ascontiguousarray(xr_all[[b for b in range(B) if b != bi]])
        in_maps.append(
            {
                "x_mine": mine,
                "x_other": other,
                "x_rest": rest,
                "w_qkvT": wqT,
                "w_oT2": woT2,
                "bn_w": bnw,
                "bn_b": bnb,
                "b_out": bo,
            }
        )
    return in_maps


def assemble(results):
    outp = np.empty((B, C, N), np.float32)
    for core in range(NCORES):
        bi, half = divmod(core, 2)
        outp[bi][:, half * NH : (half + 1) * NH] = results[core]["out"]
    return outp


def kernel(x, bn_weight, bn_bias, w_qkv, w_out, b_out):
    global _nc_cache
    if _nc_cache is None:
        _nc_cache = build()
    in_maps = make_in_maps(x, bn_weight, bn_bias, w_qkv, w_out, b_out)
    res = run_bass_kernel_spmd(_nc_cache, in_maps, list(range(NCORES)))
    return assemble(res.results)


if __name__ == "__main__":
    rng = np.random.default_rng(0)
    x = rng.standard_normal((B, C, N), dtype=np.float32)
    w_qkv = rng.standard_normal((3 * C, C), dtype=np.float32) * C**-0.5
    w_out = rng.standard_normal((C, C), dtype=np.float32) * C**-0.5
    y = kernel(
        x,
        np.ones(C, np.float32),
        np.zeros(C, np.float32),
        w_qkv,
        w_out,
        np.zeros(C, np.float32),
    )
    print(y.shape, np.abs(y).max())
